# revision 26
# baseline (speedup 1.0000x reference)
"""BusEmbedding v2: fp16 masked-slot matmul, fp16 device output.

Per token t: out[t,:] = tanh(feat[t]@W_e + b_e), e = bus_type[t].
Gate is rel-err < 2e-2, so fp16 everywhere suffices (measured ~2e-3):
no Dekker splits needed.

Geometry: pack = 1536 tokens = 128 partitions x 12 groups. Each token
owns 9 contraction slots (3 per expert: m*f0, m*f1, m with m=(bt==e)),
so slot-rows per pack = 12*9 = 108 (no padding). Host packs the input
as (bt,f0),(bt,f1),(bt,1.0) pairs so ONE scalar_tensor_tensor per
expert builds all 3 of its slots: (bt == e) * fval, cast to fp16.

Per 4-pack group: 4 PE transposes [128,108] -> [108,128] batched into
one PSUM tile + one DVE copy to SBUF (keeps the PE bubble-free), then
per pack 3 bank-aligned matmuls vs wbig [108,512] -> PSUM [128,1536]
fp32 and one ACT tanh -> fp16 SBUF; DMA out per group (1.5 MB, 3 KB
contiguous runs; per-pack DMAs on the last groups to stream the tail).
Output leaves the device as fp16 (halves HBM write traffic; tanh in
fp16 is ~5e-4) and is widened to fp32 on the host during unsharding.

The scalar engine's tanh (82 x ~1.54us) is the critical path; the PE
warmup burst exists because the HAM clock gate otherwise pins the PE
at 1.2 GHz, making the matmul chain co-critical (see comments below).
Measured: 146.4us HW exec (vs 235us session baseline), rel err 1.4e-3.
"""

import sys
from contextlib import ExitStack

import numpy as np

sys.path.insert(0, "/opt/trn_rl_repo")

import concourse.bacc as bacc  # noqa: E402
import concourse.mybir as mybir  # noqa: E402
import concourse.tile as tile  # noqa: E402
from concourse.bass_utils import run_bass_kernel_spmd  # noqa: E402

FP = mybir.dt.float32
F16 = mybir.dt.float16
D = 128
G = 12              # groups (tokens per partition per pack)
S = 9               # slots per group
KROWS = G * S       # 108 contraction rows per pack
PACK = 128 * G      # 1536 tokens per pack
NPK = 82            # packs per core
PER_CORE = NPK * PACK  # 125952
N_CORES = 8
CHUNKS = [2, 4, 8, 16, 16, 16, 20]  # build/input chunk sizes (packs)
OUT_GRP = 4         # packs per output DMA
ICOLS = G * S * 2 // 3  # 72 fp32 input cols per partition per pack
MAXC = max(CHUNKS)

_NC_CACHE = {}


def _body(ctx, tc, out, ibuf, wbig, ident):
    nc = tc.nc
    eq = mybir.AluOpType.is_equal
    mult = mybir.AluOpType.mult

    const_pool = ctx.enter_context(tc.tile_pool(name="const", bufs=1))
    ident_sb = const_pool.tile([128, 128], F16)
    nc.sync.dma_start(ident_sb[:], ident)
    wbig_sb = const_pool.tile([KROWS, 1536], F16)
    nc.sync.dma_start(wbig_sb[:], wbig)
    junk_sb = const_pool.tile([128, 128], F16)  # warmup operand
    nc.vector.memset(junk_sb[:], 0.0)

    in_pool = ctx.enter_context(tc.tile_pool(name="inp", bufs=3))
    pk_pool = ctx.enter_context(tc.tile_pool(name="pk", bufs=2))
    tp_ps = ctx.enter_context(tc.tile_pool(name="tp_ps", bufs=2, space="PSUM"))
    xsb_pool = ctx.enter_context(tc.tile_pool(name="xsb", bufs=3))
    mm_pool = ctx.enter_context(tc.tile_pool(name="mm", bufs=2, space="PSUM"))
    out_pool = ctx.enter_context(tc.tile_pool(name="outp", bufs=3))

    cstart = [0]
    for c in CHUNKS:
        cstart.append(cstart[-1] + c)
    assert cstart[-1] == NPK

    P_tiles = [None] * len(CHUNKS)

    def build(ci):
        npk = CHUNKS[ci]
        ib = in_pool.tile([128, MAXC * ICOLS], FP, tag="ib", name=f"ib{ci}")
        c0 = cstart[ci] * ICOLS
        nc.sync.dma_start(ib[:, :npk * ICOLS], ibuf[:, c0:c0 + npk * ICOLS])
        P = pk_pool.tile([128, MAXC * KROWS], F16, tag="P", name=f"P{ci}")
        P_tiles[ci] = P
        Pv = P[:, :npk * KROWS].rearrange("p (pk g s) -> p pk g s", g=G, s=S)
        iv = ib[:, :npk * ICOLS].rearrange("p (pk g j t) -> p pk g j t",
                                           g=G, j=3, t=2)
        for ei in range(3):
            nc.vector.scalar_tensor_tensor(
                Pv[:, :, :, 3 * ei:3 * ei + 3], iv[:, :, :, :, 0],
                float(ei + 1), iv[:, :, :, :, 1], op0=eq, op1=mult)

    # PE clock warmup: the HAM gate holds the PE at 1.2 GHz until it sees
    # ~5.5us of CONTINUOUS activity (HW-measured: K=8 fires then, and only
    # ~4us of full idle re-gates it). A transpose burst (~7us) on an
    # uninitialized tile — so it has no DMA dependency and starts right
    # after the framework preamble — trips the gate while the constants and
    # first input chunks stream in; per-pack bursts then keep it warm.
    for _ in range(23):
        xw = tp_ps.tile([128, 128], F16, tag="tp")
        nc.tensor.transpose(xw[:], junk_sb[:], junk_sb[:])

    build(0)
    built = 1
    for g0 in range(0, NPK, OUT_GRP):
        gpk = min(OUT_GRP, NPK - g0)
        ob = out_pool.tile([128, OUT_GRP * PACK], F16)
        # batch the group's transposes into one PSUM tile + one DVE copy:
        # the copy->transpose dependency then reaches 2 groups back, so the
        # PE runs the whole group bubble-free
        for q in range(gpk):
            pt = g0 + q
            ci = next(i for i in range(len(CHUNKS))
                      if cstart[i] <= pt < cstart[i + 1])
            while built < min(ci + 2, len(CHUNKS)):
                build(built)
                built += 1
        xps = tp_ps.tile([KROWS, OUT_GRP * 128], F16, tag="tp")
        for q in range(gpk):
            pt = g0 + q
            ci = next(i for i in range(len(CHUNKS))
                      if cstart[i] <= pt < cstart[i + 1])
            lpk = pt - cstart[ci]
            nc.tensor.transpose(xps[:, q * 128:(q + 1) * 128],
                                P_tiles[ci][:, lpk * KROWS:(lpk + 1) * KROWS],
                                ident_sb[:])
        xsb = xsb_pool.tile([KROWS, OUT_GRP * 128], F16)
        nc.vector.tensor_copy(xsb[:, :gpk * 128], xps[:, :gpk * 128])
        for q in range(gpk):
            pt = g0 + q
            mm = mm_pool.tile([128, 1536], FP)
            if pt < 8:
                # dummy matmul (overwritten by the real h=0 below) keeps
                # the PE saturated through the HAM clock-gate ramp. (Filler
                # work on later packs is a net loss: dummy matmuls slow
                # concurrent ACTIVATEs ~340ns each via PSUM/SBUF port
                # contention, and bare LDWEIGHTS don't register as HAM
                # activity — so we accept the gate re-arming mid-run.)
                nc.tensor.matmul(mm[:, 0:512], xsb[:, q * 128:(q + 1) * 128],
                                 wbig_sb[:, 0:512], start=True, stop=True)
            for h in range(3):
                nc.tensor.matmul(mm[:, h * 512:(h + 1) * 512],
                                 xsb[:, q * 128:(q + 1) * 128],
                                 wbig_sb[:, h * 512:(h + 1) * 512],
                                 start=True, stop=True)
            nc.scalar.activation(ob[:, q * PACK:(q + 1) * PACK], mm[:],
                                 mybir.ActivationFunctionType.Tanh)
        if g0 + 2 * OUT_GRP >= NPK:
            # tail: per-pack DMAs so the final writes stream out behind
            # each ACT instead of waiting for the whole group
            for q in range(gpk):
                t0 = (g0 + q) * PACK
                out_blk = out[t0:t0 + PACK, :].rearrange(
                    "(p g) d -> p (g d)", p=128, g=G)
                nc.sync.dma_start(out_blk, ob[:, q * PACK:(q + 1) * PACK])
        else:
            out_blk = out[g0 * PACK:(g0 + gpk) * PACK, :].rearrange(
                "(pk p g) d -> p pk (g d)", p=128, g=G)
            nc.sync.dma_start(out_blk,
                              ob[:, :gpk * PACK].rearrange(
                                  "p (pk gd) -> p pk gd", pk=gpk))


def build_nc():
    if 0 in _NC_CACHE:
        return _NC_CACHE[0]
    nc = bacc.Bacc("TRN2", target_bir_lowering=False, debug=False)
    ibuf = nc.dram_tensor("ibuf", [128, NPK * ICOLS], FP,
                          kind="ExternalInput").ap()
    wbig = nc.dram_tensor("wbig", [KROWS, 1536], F16,
                          kind="ExternalInput").ap()
    ident = nc.dram_tensor("ident", [128, 128], F16,
                           kind="ExternalInput").ap()
    out = nc.dram_tensor("out", [PER_CORE, D], F16,
                         kind="ExternalOutput").ap()
    with tile.TileContext(nc) as tc:
        with ExitStack() as ctx:
            _body(ctx, tc, out, ibuf, wbig, ident)
    nc.compile()
    _NC_CACHE[0] = nc
    return nc


def make_wbig(W_slack, b_slack, W_gen, b_gen, W_load, b_load):
    W_list = [np.asarray(w, np.float32) for w in (W_slack, W_gen, W_load)]
    b_list = [np.asarray(b, np.float32) for b in (b_slack, b_gen, b_load)]
    WBig = np.zeros((KROWS, 1536), np.float16)
    for g in range(G):
        col = g * 128
        base = g * S
        for ei in range(3):
            WBig[base + 3 * ei + 0, col:col + 128] = W_list[ei][0]
            WBig[base + 3 * ei + 1, col:col + 128] = W_list[ei][1]
            WBig[base + 3 * ei + 2, col:col + 128] = b_list[ei]
    return WBig


def kernel(feat, bus_type, W_slack, b_slack, W_gen, b_gen, W_load, b_load,
           **run_kwargs):
    feat = np.asarray(feat, np.float32)
    bt = np.asarray(bus_type)
    n = feat.shape[0]
    npad = N_CORES * PER_CORE
    assert n <= npad

    featp = np.zeros((npad, 2), np.float32)
    featp[:n] = feat
    btp = np.zeros(npad, np.float32)
    btp[:n] = bt.astype(np.float32)

    # device token order within a core: t = pk*1536 + p*12 + g
    bt_r = btp.reshape(N_CORES, NPK, 128, G)
    f_r = featp.reshape(N_CORES, NPK, 128, G, 2)
    ib = np.empty((N_CORES, NPK, 128, G, 3, 2), np.float32)
    ib[..., 0] = bt_r[..., None]
    ib[..., 0, 1] = f_r[..., 0]
    ib[..., 1, 1] = f_r[..., 1]
    ib[..., 2, 1] = 1.0
    ib = np.ascontiguousarray(ib.transpose(0, 2, 1, 3, 4, 5)).reshape(
        N_CORES, 128, NPK * ICOLS)

    wbig = make_wbig(W_slack, b_slack, W_gen, b_gen, W_load, b_load)
    ident = np.eye(128, dtype=np.float16)

    nc = build_nc()
    in_maps = [
        {"ibuf": ib[i], "wbig": wbig, "ident": ident}
        for i in range(N_CORES)
    ]
    try:
        res = run_bass_kernel_spmd(nc, in_maps, list(range(N_CORES)),
                                   **run_kwargs)
    except Exception:
        # A previously-failed process can leave the NeuronCores wedged
        # (NRT_EXEC_UNIT_UNRECOVERABLE); a small probe op resets them.
        import time as _time

        import jax.numpy as jnp

        for _ in range(3):
            try:
                float(jnp.sum(jnp.ones((8, 8))))
                break
            except Exception:
                _time.sleep(5)
        res = run_bass_kernel_spmd(nc, in_maps, list(range(N_CORES)),
                                   **run_kwargs)
    out = np.concatenate([res.results[i]["out"] for i in range(N_CORES)],
                         axis=0)
    kernel.last_result = res
    return out[:n].astype(np.float32)
